# revision 23
# baseline (speedup 1.0000x reference)
"""Trainium2 Bass kernel for the HNX scatter-memory block.

Sharding: 8 cores = (batch b in 0..3) x (sequence half j in 0..1).
Each core processes its 1024-token window plus W=128 warmup tokens on
each side (zero-padded at sequence edges), so both the forward and
backward EMA scans converge to the state before the window starts
(truncation error ~ sigmoid(decay)^W ~= 4e-3 relative on the scan state
for this decay range, well inside the 2e-2 gate).  No inter-core
communication.

On-chip layout is "scan layout": channels on partitions, time along the
free dimension.  All matmuls run in bf16 (1 cycle/row on the PE vs 4
for fp32); the EMA recurrences use the native DVE tensor_tensor_scan
(fp32 internal state; the decay operand stays fp32 because bf16-rounding
the decay shifts the EMA DC gain by up to ~10%).  The memory-bank read
is folded into the output projection via a host-precomputed
mem_bank @ W_out.  Per-token scalars are broadcast across partitions by
bouncing one packed row through DRAM (DMA only; gpsimd's per-op fixed
cost is reserved for the small exp(f)*f products).  Sigmoids are
computed in exp/reciprocal form so phase 3 stays on one activation
table and phase 1 on another -- no ACT_TABLE_LOAD churn.

Scheduling: phase 1 is software-pipelined (group g's in-proj matmuls
issue before group g-1's conv/scan chain) and the channel-reduction
matmuls for the entropy gate / slot softmax / memory gate accumulate
inside phase 1 as each group's forward scan lands, so the in-order PE
never drains while the per-token gate scalar chain runs.
"""

import numpy as np
import ml_dtypes
from contextlib import ExitStack

import concourse.bacc as bacc
import concourse.tile as tile
from concourse import mybir
from concourse.bass_utils import run_bass_kernel_spmd

F32 = mybir.dt.float32
BF16 = mybir.dt.bfloat16
AF = mybir.ActivationFunctionType
OP = mybir.AluOpType
BF16_NP = ml_dtypes.bfloat16


def _canon_act_tables(nc):
    """Defeat the greedy per-activation table choice: serve Exp/Ln/Identity
    from natural_log_exp_and_others and Silu from silu_and_others, then drop
    the now-redundant ACT_TABLE_LOADs (1.28us each, and they land in the
    middle of the per-token scalar chain)."""
    from concourse.hw_specs import get_activation_tables

    tables = list(get_activation_tables(nc.m.arch).items())
    prefer = [i for i, (n, _) in enumerate(tables)
              if n == "natural_log_exp_and_others"]
    prefer += [i for i, (n, _) in enumerate(tables) if n == "silu_and_others"]

    for blk in nc.main_func.blocks:
        insts = blk.instructions
        cur = None
        to_del = []
        for idx, ins in enumerate(insts):
            if isinstance(ins, mybir.InstLoadActFuncSet):
                nf = None
                for j in range(idx + 1, len(insts)):
                    if isinstance(insts[j], mybir.InstActivation):
                        nf = insts[j].func
                        break
                if nf is None:
                    continue
                cands = [i for i, (_, fs) in enumerate(tables) if nf in fs]
                tgt = next((p for p in prefer if p in cands),
                           ins.act_func_set_id)
                if cur == tgt and ins.sync_info is None:
                    to_del.append(idx)
                else:
                    ins.act_func_set_id = tgt
                    cur = tgt
            elif isinstance(ins, mybir.InstActivation):
                if cur is not None and ins.func not in tables[cur][1]:
                    raise RuntimeError(
                        f"act table patch broke {ins.func} vs {tables[cur][0]}")
        for idx in reversed(to_del):
            del insts[idx]


class Cfg:
    def __init__(self, DI=1024, H=1024, O=1024, S=128, T=2048, W=128, CH=512,
                 mm_dtype=None, sim_acts=False):
        self.DI, self.H, self.O, self.S, self.T, self.W, self.CH = DI, H, O, S, T, W, CH
        self.Tout = T // 2            # tokens per core window
        self.Tw = self.Tout + 2 * W   # work tokens per core
        self.Lw = self.Tw - W         # fwd-scan output cols (window + right warmup)
        self.KG = DI // 128           # input k-tiles
        self.HG = H // 128            # hidden channel groups
        self.OG = O // 128            # output channel groups
        self.WCH = self.Tout // CH    # phase-3 (window) chunks
        # phase-1 chunk list (col offset, width), widths <= 512 (PSUM bank)
        self.chunks = []
        off = 0
        while off < self.Tw:
            wdt = min(512, self.Tw - off)
            self.chunks.append((off, wdt))
            off += wdt
        self.sim_acts = sim_acts
        assert self.S == 128 and self.Tout % CH == 0
        assert CH <= 512 and self.W <= CH
        assert all(wd >= 256 for _, wd in self.chunks)  # bf16 1cyc needs >=256


# chp column layout: per-channel params, one column per (param, group)
CHP_NAMES = ["k0", "k1", "omdf", "df", "omdb", "db", "sbias", "ba"]
SC_F1, SC_F0, SC_NF1, SC_F2, SC_NSW, SC_NSB, SC_NBMG = range(7)


def build_program(cfg: Cfg):
    c = cfg
    nc = bacc.Bacc("TRN2", target_bir_lowering=False, debug=False,
                   enable_asserts=False)

    xt = nc.dram_tensor("xt", [c.DI, c.Tw], BF16, kind="ExternalInput").ap()
    w_in = nc.dram_tensor("w_in", [c.DI, 2 * c.H], BF16, kind="ExternalInput").ap()
    w_out = nc.dram_tensor("w_out", [c.H, c.O], BF16, kind="ExternalInput").ap()
    w_slot = nc.dram_tensor("w_slot", [c.H, c.S], BF16, kind="ExternalInput").ap()
    w_mg = nc.dram_tensor("w_mg", [c.H, 1], BF16, kind="ExternalInput").ap()
    wm_d = nc.dram_tensor("wm", [c.S, c.O], BF16, kind="ExternalInput").ap()
    chp = nc.dram_tensor("chp", [128, len(CHP_NAMES) * c.HG], F32,
                         kind="ExternalInput").ap()
    bout_d = nc.dram_tensor("bout", [128, c.OG], F32, kind="ExternalInput").ap()
    bslot_d = nc.dram_tensor("bslot", [128, 1], F32, kind="ExternalInput").ap()
    sc = nc.dram_tensor("sc", [1, 8], F32, kind="ExternalInput").ap()
    mask_d = nc.dram_tensor("mask", [1, c.W], BF16, kind="ExternalInput").ap()
    out_d = nc.dram_tensor("outT", [c.O, c.Tout], F32, kind="ExternalOutput").ap()
    # DRAM bounce rows for cross-partition broadcast of per-token scalars
    rowbc = nc.dram_tensor("rowbc", [c.WCH, 2 * c.CH], BF16,
                           kind="Internal").ap()

    with tile.TileContext(nc) as tc:
        with ExitStack() as top:
            consts = top.enter_context(tc.tile_pool(name="consts", bufs=1))
            chp_t = consts.tile([128, len(CHP_NAMES) * c.HG], F32)
            sc_t = consts.tile([1, 8], F32)
            scB = consts.tile([128, 8], F32)
            bout_t = consts.tile([128, c.OG], F32)
            bslot_t = consts.tile([128, 1], F32)
            ones_b = consts.tile([128, 1], BF16)
            mbw = consts.tile([128, c.W], BF16)
            wslot_t = [consts.tile([128, c.S], BF16, name=f"ws{k}", tag=f"ws{k}")
                       for k in range(c.HG)]
            wmg_t = [consts.tile([128, 1], BF16, name=f"wmg{k}", tag=f"wmg{k}")
                     for k in range(c.HG)]
            wm_t = consts.tile([128, c.O], BF16)
            wout_t = [consts.tile([128, c.O], BF16, name=f"wo{k}", tag=f"wo{k}")
                      for k in range(c.HG)]

            fpool = top.enter_context(tc.tile_pool(name="f", bufs=1))
            f_all = fpool.tile([128, c.HG * c.Lw], BF16)
            gb_all = fpool.tile([128, c.HG * c.Tout], BF16)
            # exp(f) and f*exp(f) over the output window, filled per-g as
            # each forward scan lands
            pt_all = [fpool.tile([128, c.HG * c.CH], BF16, name=f"ptw{w}",
                                 tag=f"ptw{w}") for w in range(c.WCH)]
            pft_all = [fpool.tile([128, c.HG * c.CH], BF16, name=f"pfw{w}",
                                  tag=f"pfw{w}") for w in range(c.WCH)]

            # entropy/slot/memory-gate PSUM accumulators live across all of
            # phase 1 (pZ/pG/pM share one bank at partitions 0/32/64)
            psE = top.enter_context(tc.tile_pool(name="psE", bufs=2,
                                                 space="PSUM"))
            pR_t = [psE.tile([65, c.CH], F32, tag="pR", name=f"pR{w}")
                    for w in range(c.WCH)]
            pL_t = [psE.tile([128, c.CH], F32, tag="pL", name=f"pL{w}")
                    for w in range(c.WCH)]

            # phase-1-only tiles; released before the phase-3 tiles allocate
            p1 = top.enter_context(ExitStack())
            xw_pool = p1.enter_context(tc.tile_pool(name="xw", bufs=1))
            xt_t = [xw_pool.tile([128, c.Tw], BF16, name=f"xt{k}", tag=f"xt{k}")
                    for k in range(c.KG)]
            wag_t = [xw_pool.tile([128, 128 * c.KG], BF16, name=f"wa{g}",
                                  tag=f"wa{g}") for g in range(c.HG)]
            wdg_t = [xw_pool.tile([128, 128 * c.KG], BF16, name=f"wd{g}",
                                  tag=f"wd{g}") for g in range(c.HG)]

            def load_wg(g):
                nc.sync.dma_start(
                    wag_t[g][:].rearrange("p (k m) -> p k m", m=128),
                    w_in[:, g * 128:(g + 1) * 128]
                    .rearrange("(k p) m -> p k m", p=128))
                nc.sync.dma_start(
                    wdg_t[g][:].rearrange("p (k m) -> p k m", m=128),
                    w_in[:, c.H + g * 128:c.H + (g + 1) * 128]
                    .rearrange("(k p) m -> p k m", p=128))

            # DMA priority: the exact inputs of the first matmuls first
            for n, (off, wdt) in enumerate(c.chunks):
                for k in range(c.KG):
                    nc.sync.dma_start(xt_t[k][:, off:off + wdt],
                                      xt[k * 128:(k + 1) * 128, off:off + wdt])
                if n < 2:
                    load_wg(n)
            for g in range(2, c.HG):
                load_wg(g)

            # background loads (phase-3 weights, per-channel params)
            nc.sync.dma_start(chp_t[:], chp[:])
            nc.sync.dma_start(sc_t[:], sc[:])
            nc.sync.dma_start(scB[:], sc.broadcast_to([128, 8]))
            nc.sync.dma_start(bout_t[:], bout_d[:])
            nc.sync.dma_start(bslot_t[:], bslot_d[:])
            nc.vector.memset(ones_b[:], 1.0)
            nc.sync.dma_start(mbw[:], mask_d.broadcast_to([128, c.W]))
            for k in range(c.HG):
                nc.sync.dma_start(wslot_t[k][:], w_slot[k * 128:(k + 1) * 128, :])
            for k in range(c.HG):
                nc.sync.dma_start(wmg_t[k][:], w_mg[k * 128:(k + 1) * 128, :])
            nc.sync.dma_start(wm_t[:], wm_d[:])
            for k in range(c.HG):
                nc.sync.dma_start(wout_t[k][:], w_out[k * 128:(k + 1) * 128, :])

            def chpc(name, g):
                i = CHP_NAMES.index(name) * c.HG + g
                return chp_t[:, i:i + 1]

            def scc(i):
                return sc_t[0:1, i:i + 1]

            def sccB(i):  # per-partition copy for [128,*] scalar operands
                return scB[:, i:i + 1]

            # -------- phase 1+2: in-proj, conv, scans, channel reductions --
            # Software-pipelined: group g's in-proj matmuls and PSUM-draining
            # ops issue BEFORE group g-1's conv/scan chain, so the in-order
            # DVE serves x1 writes promptly and the PE never waits on PSUM.
            pch = p1.enter_context(tc.tile_pool(name="pch", bufs=3))
            scr = p1.enter_context(tc.tile_pool(name="scr", bufs=2))
            ps1 = p1.enter_context(tc.tile_pool(name="ps1", bufs=2,
                                                space="PSUM"))

            def inproj(g):
                ga, gd = g * 128, c.H + g * 128
                x1p = pch.tile([128, c.Tw + 1], BF16, tag="x1p",
                               name=f"x1p{g}")
                nc.vector.memset(x1p[:, 0:1], 0.0)
                for off, wdt in c.chunks:
                    pa = ps1.tile([128, wdt], F32, tag="pa")
                    pdt = ps1.tile([128, wdt], F32, tag="pdt")
                    for k in range(c.KG):
                        nc.tensor.matmul(
                            pa[:], wag_t[g][:, k * 128:(k + 1) * 128],
                            xt_t[k][:, off:off + wdt],
                            start=(k == 0), stop=(k == c.KG - 1))
                    for k in range(c.KG):
                        nc.tensor.matmul(
                            pdt[:], wdg_t[g][:, k * 128:(k + 1) * 128],
                            xt_t[k][:, off:off + wdt],
                            start=(k == 0), stop=(k == c.KG - 1))
                    sdt = scr.tile([128, wdt], F32, tag="sdt")
                    nc.scalar.activation(sdt[:], pdt[:], AF.Silu,
                                         bias=chpc("sbias", g))
                    # x1 = (a + b_a) * silu(dt + sbias)
                    nc.vector.scalar_tensor_tensor(
                        x1p[:, 1 + off:1 + off + wdt], pa[:],
                        chpc("ba", g), sdt[:], OP.add, OP.mult)
                return x1p

            def convscan(g, x1p):
                # causal depthwise conv k=2 + silu (+ (1-d) prescale)
                sx = pch.tile([128, c.Tw], BF16, tag="sx")
                nc.vector.tensor_scalar(sx[:], x1p[:, 1:], chpc("k1", g),
                                        None, OP.mult)
                ypre = pch.tile([128, c.Tw], BF16, tag="ypre")
                nc.vector.scalar_tensor_tensor(
                    ypre[:], x1p[:, 0:c.Tw], chpc("k0", g), sx[:],
                    OP.mult, OP.add)
                ysl = pch.tile([128, c.Tw], BF16, tag="ysl")
                nc.scalar.activation(ysl[:], ypre[:], AF.Silu)
                u = pch.tile([128, c.Tw], BF16, tag="sx")
                nc.scalar.mul(u[:], ysl[:], chpc("omdf", g))

                # fwd EMA scan; first W tokens into discard scratch
                fsl = f_all[:, g * c.Lw:(g + 1) * c.Lw]
                dfb_w = chpc("df", g).broadcast_to([128, c.W])
                dfb_m = chpc("df", g).broadcast_to([128, c.Lw])
                fscr = scr.tile([128, c.W], F32, tag="fscr")
                nc.vector.tensor_tensor_scan(
                    fscr[:], dfb_w, u[:, 0:c.W], 0.0, OP.mult, OP.add)
                nc.vector.tensor_tensor_scan(
                    fsl, dfb_m, u[:, c.W:c.Tw],
                    fscr[:, c.W - 1:c.W], OP.mult, OP.add)

                # entropy/slot/memory-gate contributions of this group, for
                # both token chunks; accumulate into the phase-spanning PSUM
                st, sp = (g == 0), (g == c.HG - 1)
                for w in range(c.WCH):
                    gsl = slice(g * c.CH, (g + 1) * c.CH)
                    fwsl = f_all[:, g * c.Lw + w * c.CH:
                                 g * c.Lw + (w + 1) * c.CH]
                    nc.scalar.activation(pt_all[w][:, gsl], fwsl, AF.Exp)
                    nc.gpsimd.tensor_tensor(pft_all[w][:, gsl],
                                            pt_all[w][:, gsl], fwsl,
                                            OP.mult)
                    pR, pL = pR_t[w], pL_t[w]
                    nc.tensor.matmul(pR[0:1, :], ones_b[:],
                                     pt_all[w][:, gsl], start=st, stop=sp)
                    nc.tensor.matmul(pR[32:33, :], ones_b[:],
                                     pft_all[w][:, gsl], start=st, stop=sp)
                    nc.tensor.matmul(pR[64:65, :], wmg_t[g][:],
                                     fwsl, start=st, stop=sp)
                    nc.tensor.matmul(pL[:], wslot_t[g][:],
                                     fwsl, start=st, stop=sp)

                # bwd scan (phase 2, interleaved): warmup slice masked
                gsl = gb_all[:, g * c.Tout:(g + 1) * c.Tout]
                dbb_w = chpc("db", g).broadcast_to([128, c.W])
                dbb_m = chpc("db", g).broadcast_to([128, c.Tout])
                d1m = scr.tile([128, c.W], BF16, tag="d1m")
                nc.vector.scalar_tensor_tensor(
                    d1m[:], fsl[:, c.Tout:c.Lw], chpc("omdb", g),
                    mbw[:], OP.mult, OP.mult)
                d1w = pch.tile([128, c.Tout], BF16, tag="ypre")
                nc.scalar.mul(d1w[:], fsl[:, 0:c.Tout], chpc("omdb", g))
                bscr = scr.tile([128, c.W], F32, tag="bscr")
                nc.vector.tensor_tensor_scan(
                    bscr[:, ::-1], dbb_w, d1m[:, ::-1],
                    0.0, OP.mult, OP.add)
                nc.vector.tensor_tensor_scan(
                    gsl[:, ::-1], dbb_m, d1w[:, ::-1],
                    bscr[:, 0:1], OP.mult, OP.add)

            prev = None
            for g in range(c.HG):
                x1p = inproj(g)
                if prev is not None:
                    convscan(prev[0], prev[1])
                prev = (g, x1p)
            convscan(prev[0], prev[1])

            p1.close()  # release xt/w_in/conv scratch before phase-3 tiles

            # ------------- phase 3: gate scalars, fusion, out-proj ------
            with ExitStack() as p2:
                p3 = p2.enter_context(tc.tile_pool(name="p3", bufs=2))
                pb1 = p2.enter_context(tc.tile_pool(name="pb1", bufs=2))
                row = p2.enter_context(tc.tile_pool(name="row", bufs=2))
                psO = p2.enter_context(tc.tile_pool(name="psO", bufs=3,
                                                    space="PSUM"))

                fv = f_all[:].rearrange("p (g l) -> p g l", l=c.Lw)
                gv = gb_all[:].rearrange("p (g l) -> p g l", l=c.Tout)

                # pass P(w): per-token gate scalars + fusion
                wt_t, E2_t = {}, {}
                for w in range(c.WCH):
                    sl = slice(w * c.CH, (w + 1) * c.CH)
                    fw = fv[:, :, sl]       # [128, HG, CH] window view
                    gw = gv[:, :, sl]
                    pR, pL = pR_t[w], pL_t[w]
                    pZ, pG, pM = pR[0:1, :], pR[32:33, :], pR[64:65, :]

                    E = p3.tile([128, c.CH], BF16, tag="E")
                    nc.scalar.activation(E[:], pL[:], AF.Exp, bias=bslot_t[:])
                    # reuse row 0 of the (now dead) slot-logit bank for Zs
                    pZs = pL[0:1, :]
                    nc.tensor.matmul(pZs, ones_b[:], E[:],
                                     start=True, stop=True)

                    # f - gb for the gate-scaled contrast term; no row deps
                    D = p3.tile([128, c.HG * c.CH], BF16, tag="D")
                    Dv = D[:].rearrange("p (g l) -> p g l", l=c.CH)
                    nc.vector.tensor_tensor(Dv, fw, gw, OP.subtract)

                    # per-token gate scalars ([1, CH] rows); sigmoids in
                    # exp/recip form to stay on the natural_log_exp table
                    Zr = row.tile([1, c.CH], F32, tag="Zr")
                    nc.vector.reciprocal_approx_fast(Zr[:], pZ)
                    lnZ = row.tile([1, c.CH], F32, tag="lnZ")
                    nc.scalar.activation(lnZ[:], pZ, AF.Ln)
                    gz = row.tile([1, c.CH], F32, tag="gz")
                    nc.vector.tensor_tensor(gz[:], pG, Zr[:], OP.mult)
                    ent = row.tile([1, c.CH], F32, tag="ent")
                    nc.vector.tensor_tensor(ent[:], lnZ[:], gz[:], OP.subtract)
                    sgq = row.tile([1, c.CH], F32, tag="sgq")
                    nc.scalar.activation(sgq[:], ent[:], AF.Exp,
                                         scale=scc(SC_NSW), bias=scc(SC_NSB))
                    sg1 = row.tile([1, c.CH], F32, tag="sg1")
                    nc.vector.tensor_scalar(sg1[:], sgq[:], 1.0, None, OP.add)
                    gate = row.tile([1, c.CH], F32, tag="gate")
                    nc.vector.reciprocal_approx_fast(gate[:], sg1[:])
                    mgq = row.tile([1, c.CH], F32, tag="mgq")
                    nc.scalar.activation(mgq[:], pM, AF.Exp,
                                         scale=-1.0, bias=scc(SC_NBMG))
                    mg1 = row.tile([1, c.CH], F32, tag="mg1")
                    nc.vector.tensor_scalar(mg1[:], mgq[:], 1.0, None, OP.add)
                    mgi = row.tile([1, c.CH], F32, tag="mgi")
                    nc.vector.reciprocal_approx_fast(mgi[:], mg1[:])
                    Zsr = row.tile([1, c.CH], F32, tag="Zsr")
                    nc.vector.reciprocal_approx_fast(Zsr[:], pZs)
                    # pack GB=f1*gate | s2 into one row, bounce via DRAM to
                    # broadcast across partitions (DMA only)
                    abc = row.tile([1, 2 * c.CH], BF16, tag="abc")
                    nc.vector.tensor_scalar(abc[:, 0:c.CH], gate[:],
                                            scc(SC_F1), None, OP.mult)
                    nc.vector.scalar_tensor_tensor(
                        abc[:, c.CH:2 * c.CH], mgi[:], scc(SC_F2),
                        Zsr[:], OP.mult, OP.mult)
                    nc.sync.dma_start(rowbc[w:w + 1, :], abc[:])
                    ABC = pb1.tile([128, 2 * c.CH], BF16, tag="ABC")
                    nc.sync.dma_start(
                        ABC[:], rowbc[w:w + 1, :].broadcast_to([128, 2 * c.CH]))
                    GBB = ABC[:, 0:c.CH]
                    S2B = ABC[:, c.CH:2 * c.CH]

                    E2 = p3.tile([128, c.CH], BF16, tag="E2")
                    nc.vector.tensor_tensor(E2[:], E[:], S2B, OP.mult)
                    E2_t[w] = E2

                    # weighted = f0*f + (f1*gate)*(f - gb), in halves so the
                    # out-proj k-loop can start on the first half; memory
                    # read is folded into out-proj
                    GBv = GBB.rearrange("p (o l) -> p o l", o=1) \
                        .broadcast_to([128, c.HG // 2, c.CH])
                    wtp = p3.tile([128, c.HG * c.CH], BF16, tag="wtp")
                    wtpv = wtp[:].rearrange("p (g l) -> p g l", l=c.CH)
                    wt = p3.tile([128, c.HG * c.CH], BF16, tag="wt")
                    wtv = wt[:].rearrange("p (g l) -> p g l", l=c.CH)
                    hh = c.HG // 2
                    for h in (slice(0, hh), slice(hh, c.HG)):
                        nc.vector.tensor_tensor(wtpv[:, h], Dv[:, h], GBv,
                                                OP.mult)
                        nc.vector.scalar_tensor_tensor(
                            wtv[:, h], fw[:, h], sccB(SC_F0), wtpv[:, h],
                            OP.mult, OP.add)
                    wt_t[w] = wt

                # pass Q(w): out-proj (+ folded memory read, WM last so E2
                # has time to land while the wout accumulation streams)
                for w in range(c.WCH):
                    sl = slice(w * c.CH, (w + 1) * c.CH)
                    wt, E2 = wt_t[w], E2_t[w]
                    for m in range(c.OG):
                        mc = slice(m * 128, (m + 1) * 128)
                        po = psO.tile([128, c.CH], F32, tag="po")
                        for k in range(c.HG):
                            nc.tensor.matmul(
                                po[:], wout_t[k][:, mc],
                                wt[:, k * c.CH:(k + 1) * c.CH],
                                start=(k == 0), stop=False)
                        nc.tensor.matmul(po[:], wm_t[:, mc], E2[:],
                                         start=False, stop=True)
                        ob = p3.tile([128, c.CH], F32, tag="ob")
                        nc.scalar.activation(ob[:], po[:], AF.Identity,
                                             bias=bout_t[:, m:m + 1])
                        nc.sync.dma_start(out_d[m * 128:(m + 1) * 128, sl], ob[:])

    nc.compile()
    _canon_act_tables(nc)
    return nc


_PROG_CACHE = {}


def _get_prog(cfg: Cfg):
    key = (cfg.DI, cfg.H, cfg.O, cfg.S, cfg.T, cfg.W, cfg.CH)
    if key not in _PROG_CACHE:
        _PROG_CACHE[key] = build_program(cfg)
    return _PROG_CACHE[key]


def make_in_maps(cfg, x, W_in, b_in, dt_bias_fwd, conv_k, decay_fwd, decay_bwd,
                 memory, mem_decay, W_mem_gate, b_mem_gate, W_slot, b_slot,
                 W_slot_bwd, b_slot_bwd, fusion_weight, scaler_w, scaler_b,
                 W_out, b_out):
    c = cfg
    x = np.asarray(x)
    B, T, DI = x.shape
    f32 = np.float32

    def sig(v):
        return 1.0 / (1.0 + np.exp(-np.asarray(v, np.float64)))

    def col(v):  # [H] -> [128, HG] column blocks
        return np.ascontiguousarray(np.asarray(v, f32).reshape(c.HG, 128).T)

    df = sig(decay_fwd)
    db = sig(decay_bwd)
    chp = np.concatenate([
        col(conv_k[:, 0]), col(conv_k[:, 1]),
        col((1.0 - df)), col(df),
        col((1.0 - db)), col(db),
        col(np.asarray(b_in)[c.H:] + np.asarray(dt_bias_fwd)),
        col(np.asarray(b_in)[:c.H]),
    ], axis=1).astype(f32)
    bout = np.ascontiguousarray(np.asarray(b_out, f32).reshape(c.OG, 128).T)
    bslot = np.asarray(b_slot_bwd, f32).reshape(128, 1)
    scv = np.zeros((1, 8), f32)
    scv[0, SC_F1] = fusion_weight[1]
    scv[0, SC_F0] = fusion_weight[0]
    scv[0, SC_NF1] = -fusion_weight[1]
    scv[0, SC_F2] = fusion_weight[2]
    scv[0, SC_NSW] = -scaler_w[0]
    scv[0, SC_NSB] = -scaler_b[0]
    scv[0, SC_NBMG] = -b_mem_gate[0]
    mem_scaled = np.asarray(memory, f32) * sig(mem_decay)[:, None].astype(f32)
    wm = (mem_scaled.astype(f32) @ np.asarray(W_out, f32)).astype(BF16_NP)

    shared = {
        "w_in": np.ascontiguousarray(np.asarray(W_in, f32).astype(BF16_NP)),
        "w_out": np.ascontiguousarray(np.asarray(W_out, f32).astype(BF16_NP)),
        "w_slot": np.ascontiguousarray(np.asarray(W_slot_bwd, f32).astype(BF16_NP)),
        "w_mg": np.ascontiguousarray(np.asarray(W_mem_gate, f32).astype(BF16_NP)),
        "wm": np.ascontiguousarray(wm),
        "chp": chp, "bout": bout, "bslot": bslot, "sc": scv,
    }
    in_maps = []
    for core in range(8):
        b, j = divmod(core, 2)
        start = j * c.Tout - c.W
        gs, ge = max(0, start), min(T, start + c.Tw)
        xt = np.zeros((c.DI, c.Tw), BF16_NP)
        xt[:, gs - start:ge - start] = x[b, gs:ge, :].T.astype(BF16_NP)
        # mask for the bwd-scan warmup slice (local tokens [W+Tout, Tw)):
        # 1 where the token is a real sequence position, 0 in the zero-pad
        mask = np.zeros((1, c.W), BF16_NP)
        ws, we = c.W + c.Tout, c.Tw  # local token range of the warmup slice
        vs, ve = gs - start, ge - start  # valid local token range
        ov_s, ov_e = max(ws, vs), min(we, ve)
        if ov_s < ov_e:
            mask[0, ov_s - ws:ov_e - ws] = 1.0
        m = dict(shared)
        m["xt"] = xt
        m["mask"] = mask
        in_maps.append(m)
    return in_maps


def run(cfg, inputs, trace=False, tmpdir=None):
    nc = _get_prog(cfg)
    in_maps = make_in_maps(cfg, **inputs)
    res = run_bass_kernel_spmd(nc, in_maps, core_ids=list(range(8)),
                               trace=trace, tmpdir=tmpdir)
    B, T = np.asarray(inputs["x"]).shape[0], np.asarray(inputs["x"]).shape[1]
    out = np.empty((B, T, cfg.O), np.float32)
    for core in range(8):
        b, j = divmod(core, 2)
        out[b, j * cfg.Tout:(j + 1) * cfg.Tout, :] = res.results[core]["outT"].T
    return out, res


def kernel(**inputs):
    cfg = Cfg()
    out, _ = run(cfg, inputs)
    return out


# revision 29
# speedup vs baseline: 1.1437x; 1.1437x over previous
"""Trainium2 Bass kernel for the HNX scatter-memory block.

Sharding: 8 cores = (batch b in 0..3) x (sequence half j in 0..1).
Each core processes its 1024-token window plus W=128 warmup tokens on
each side (zero-padded at sequence edges), so both the forward and
backward EMA scans converge to the state before the window starts
(truncation error ~ sigmoid(decay)^W ~= 4e-3 relative on the scan state
for this decay range, well inside the 2e-2 gate).  No inter-core
communication.

On-chip layout is "scan layout": channels on partitions, time along the
free dimension.  All matmuls run in bf16 (1 cycle/row on the PE vs 4
for fp32); the EMA recurrences use the native DVE tensor_tensor_scan
(fp32 internal state; the decay operand stays fp32 because bf16-rounding
the decay shifts the EMA DC gain by up to ~10%).  The memory-bank read
is folded into the output projection via a host-precomputed
mem_bank @ W_out.  Per-token scalars are broadcast across partitions by
bouncing one packed row through DRAM (DMA only; gpsimd's per-op fixed
cost is reserved for the small exp(f)*f products).  Sigmoids are
computed in exp/reciprocal form so phase 3 stays on one activation
table and phase 1 on another -- no ACT_TABLE_LOAD churn.

Scheduling: phase 1 is software-pipelined (group g's in-proj matmuls
issue before group g-1's conv/scan chain) and the channel-reduction
matmuls for the entropy gate / slot softmax / memory gate accumulate
inside phase 1 as each group's forward scan lands, so the in-order PE
never drains while the per-token gate scalar chain runs.
"""

import numpy as np
import ml_dtypes
from contextlib import ExitStack

import concourse.bacc as bacc
import concourse.tile as tile
from concourse import mybir
from concourse.bass_utils import run_bass_kernel_spmd

F32 = mybir.dt.float32
BF16 = mybir.dt.bfloat16
AF = mybir.ActivationFunctionType
OP = mybir.AluOpType
BF16_NP = ml_dtypes.bfloat16


def _canon_act_tables(nc):
    """Defeat the greedy per-activation table choice: serve Exp/Ln/Identity
    from natural_log_exp_and_others and Silu from silu_and_others, then drop
    the now-redundant ACT_TABLE_LOADs (1.28us each, and they land in the
    middle of the per-token scalar chain)."""
    from concourse.hw_specs import get_activation_tables

    tables = list(get_activation_tables(nc.m.arch).items())
    prefer = [i for i, (n, _) in enumerate(tables)
              if n == "natural_log_exp_and_others"]
    prefer += [i for i, (n, _) in enumerate(tables) if n == "silu_and_others"]

    for blk in nc.main_func.blocks:
        insts = blk.instructions
        cur = None
        to_del = []
        for idx, ins in enumerate(insts):
            if isinstance(ins, mybir.InstLoadActFuncSet):
                nf = None
                for j in range(idx + 1, len(insts)):
                    if isinstance(insts[j], mybir.InstActivation):
                        nf = insts[j].func
                        break
                if nf is None:
                    continue
                cands = [i for i, (_, fs) in enumerate(tables) if nf in fs]
                tgt = next((p for p in prefer if p in cands),
                           ins.act_func_set_id)
                if cur == tgt and ins.sync_info is None:
                    to_del.append(idx)
                else:
                    ins.act_func_set_id = tgt
                    cur = tgt
            elif isinstance(ins, mybir.InstActivation):
                if cur is not None and ins.func not in tables[cur][1]:
                    raise RuntimeError(
                        f"act table patch broke {ins.func} vs {tables[cur][0]}")
        for idx in reversed(to_del):
            del insts[idx]


class Cfg:
    def __init__(self, DI=1024, H=1024, O=1024, S=128, T=2048, W=128, CH=512,
                 mm_dtype=None, sim_acts=False):
        self.DI, self.H, self.O, self.S, self.T, self.W, self.CH = DI, H, O, S, T, W, CH
        self.Tout = T // 2            # tokens per core window
        self.Tw = self.Tout + 2 * W   # work tokens per core
        self.Lw = self.Tw - W         # fwd-scan output cols (window + right warmup)
        self.KG = DI // 128           # input k-tiles
        self.HG = H // 128            # hidden channel groups
        self.OG = O // 128            # output channel groups
        self.WCH = self.Tout // CH    # phase-3 (window) chunks
        # phase-1 chunk list (col offset, width), widths <= 512 (PSUM bank)
        self.chunks = []
        off = 0
        while off < self.Tw:
            wdt = min(512, self.Tw - off)
            self.chunks.append((off, wdt))
            off += wdt
        self.sim_acts = sim_acts
        assert self.S == 128 and self.Tout % CH == 0
        assert CH <= 512 and self.W <= CH
        assert all(wd >= 256 for _, wd in self.chunks)  # bf16 1cyc needs >=256


# chp column layout: per-channel params, one column per (param, group)
CHP_NAMES = ["k0", "k1", "omdf", "df", "omdb", "db", "sbias", "ba"]
SC_F1, SC_F0, SC_NF1, SC_F2, SC_NSW, SC_NSB, SC_NBMG = range(7)


def build_program(cfg: Cfg):
    c = cfg
    nc = bacc.Bacc("TRN2", target_bir_lowering=False, debug=False,
                   enable_asserts=False)

    xt = nc.dram_tensor("xt", [c.DI, c.Tw], BF16, kind="ExternalInput").ap()
    w_in = nc.dram_tensor("w_in", [c.DI, 2 * c.H], BF16, kind="ExternalInput").ap()
    w_out = nc.dram_tensor("w_out", [c.H, c.O], BF16, kind="ExternalInput").ap()
    w_slot = nc.dram_tensor("w_slot", [c.H, c.S], BF16, kind="ExternalInput").ap()
    w_mg = nc.dram_tensor("w_mg", [c.H, 1], BF16, kind="ExternalInput").ap()
    wm_d = nc.dram_tensor("wm", [c.S, c.O], BF16, kind="ExternalInput").ap()
    chp = nc.dram_tensor("chp", [128, len(CHP_NAMES) * c.HG], F32,
                         kind="ExternalInput").ap()
    bout_d = nc.dram_tensor("bout", [128, c.OG], F32, kind="ExternalInput").ap()
    bslot_d = nc.dram_tensor("bslot", [128, 1], F32, kind="ExternalInput").ap()
    sc = nc.dram_tensor("sc", [1, 8], F32, kind="ExternalInput").ap()
    mask_d = nc.dram_tensor("mask", [1, c.W], BF16, kind="ExternalInput").ap()
    out_d = nc.dram_tensor("outT", [c.O, c.Tout], F32, kind="ExternalOutput").ap()
    # DRAM bounce rows for cross-partition broadcast of per-token scalars
    rowbc = nc.dram_tensor("rowbc", [c.WCH, 2 * c.CH], BF16,
                           kind="Internal").ap()

    with tile.TileContext(nc) as tc:
        with ExitStack() as top:
            consts = top.enter_context(tc.tile_pool(name="consts", bufs=1))
            chp_t = consts.tile([128, len(CHP_NAMES) * c.HG], F32)
            sc_t = consts.tile([1, 8], F32)
            scB = consts.tile([128, 8], F32)
            bout_t = consts.tile([128, c.OG], F32)
            bslot_t = consts.tile([128, 1], F32)
            ones_b = consts.tile([128, 1], BF16)
            mbw = consts.tile([128, c.W], BF16)
            wslot_t = [consts.tile([128, c.S], BF16, name=f"ws{k}", tag=f"ws{k}")
                       for k in range(c.HG)]
            wmg_t = [consts.tile([128, 1], BF16, name=f"wmg{k}", tag=f"wmg{k}")
                     for k in range(c.HG)]
            wm_t = consts.tile([128, c.O], BF16)
            wout_t = [consts.tile([128, c.O], BF16, name=f"wo{k}", tag=f"wo{k}")
                      for k in range(c.HG)]

            fpool = top.enter_context(tc.tile_pool(name="f", bufs=1))
            f_all = fpool.tile([128, c.HG * c.Lw], BF16)
            gb_all = fpool.tile([128, c.HG * c.Tout], BF16)
            # exp(f) and f*exp(f) over the output window, filled per-g as
            # each forward scan lands
            pt_all = [fpool.tile([128, c.HG * c.CH], BF16, name=f"ptw{w}",
                                 tag=f"ptw{w}") for w in range(c.WCH)]
            pft_all = [fpool.tile([128, c.HG * c.CH], BF16, name=f"pfw{w}",
                                  tag=f"pfw{w}") for w in range(c.WCH)]

            # entropy/slot/memory-gate PSUM accumulators for token-chunk 0
            # live across all of phase 1 (pZ/pG/pM share one bank at
            # partitions 0/32/64); chunk 1's accumulate right after the
            # g-loop from the phase-3 PSUM pool
            psE = top.enter_context(tc.tile_pool(name="psE", bufs=1,
                                                 space="PSUM"))
            pR_t = {0: psE.tile([65, c.CH], F32, tag="pR", name="pR0")}
            pL_t = {0: psE.tile([128, c.CH], F32, tag="pL", name="pL0")}

            # phase-1-only tiles; released before the phase-3 tiles allocate
            p1 = top.enter_context(ExitStack())
            xw_pool = p1.enter_context(tc.tile_pool(name="xw", bufs=1))
            xt_t = [xw_pool.tile([128, c.Tw], BF16, name=f"xt{k}", tag=f"xt{k}")
                    for k in range(c.KG)]
            wag_t = [xw_pool.tile([128, 128 * c.KG], BF16, name=f"wa{g}",
                                  tag=f"wa{g}") for g in range(c.HG)]
            wdg_t = [xw_pool.tile([128, 128 * c.KG], BF16, name=f"wd{g}",
                                  tag=f"wd{g}") for g in range(c.HG)]

            def load_wg(g):
                nc.sync.dma_start(
                    wag_t[g][:].rearrange("p (k m) -> p k m", m=128),
                    w_in[:, g * 128:(g + 1) * 128]
                    .rearrange("(k p) m -> p k m", p=128))
                nc.sync.dma_start(
                    wdg_t[g][:].rearrange("p (k m) -> p k m", m=128),
                    w_in[:, c.H + g * 128:c.H + (g + 1) * 128]
                    .rearrange("(k p) m -> p k m", p=128))

            # DMA priority: tiny parameter tables first (they gate the Act
            # engine's first op), then the first matmuls' exact inputs
            nc.sync.dma_start(chp_t[:], chp[:])
            nc.sync.dma_start(sc_t[:], sc[:])
            nc.sync.dma_start(scB[:], sc.broadcast_to([128, 8]))
            nc.sync.dma_start(bout_t[:], bout_d[:])
            nc.sync.dma_start(bslot_t[:], bslot_d[:])
            nc.vector.memset(ones_b[:], 1.0)
            nc.sync.dma_start(mbw[:], mask_d.broadcast_to([128, c.W]))
            for k in range(c.HG):
                nc.sync.dma_start(wslot_t[k][:], w_slot[k * 128:(k + 1) * 128, :])
            for k in range(c.HG):
                nc.sync.dma_start(wmg_t[k][:], w_mg[k * 128:(k + 1) * 128, :])
            for n, (off, wdt) in enumerate(c.chunks):
                for k in range(c.KG):
                    nc.sync.dma_start(xt_t[k][:, off:off + wdt],
                                      xt[k * 128:(k + 1) * 128, off:off + wdt])
                if n < 2:
                    load_wg(n)
            for g in range(2, c.HG):
                load_wg(g)

            # background loads (phase-3 weights)
            nc.sync.dma_start(wm_t[:], wm_d[:])
            for k in range(c.HG):
                nc.sync.dma_start(wout_t[k][:], w_out[k * 128:(k + 1) * 128, :])

            def chpc(name, g):
                i = CHP_NAMES.index(name) * c.HG + g
                return chp_t[:, i:i + 1]

            def scc(i):
                return sc_t[0:1, i:i + 1]

            def sccB(i):  # per-partition copy for [128,*] scalar operands
                return scB[:, i:i + 1]

            # -------- phase 1+2: in-proj, conv, scans, channel reductions --
            # Software-pipelined: group g's in-proj matmuls and PSUM-draining
            # ops issue BEFORE group g-1's conv/scan chain, so the in-order
            # DVE serves x1 writes promptly and the PE never waits on PSUM.
            pch = p1.enter_context(tc.tile_pool(name="pch", bufs=3))
            scr = p1.enter_context(tc.tile_pool(name="scr", bufs=2))
            ps1 = p1.enter_context(tc.tile_pool(name="ps1", bufs=3,
                                                space="PSUM"))

            def inproj(g):
                ga, gd = g * 128, c.H + g * 128
                x1p = pch.tile([128, c.Tw + 1], BF16, tag="x1p",
                               name=f"x1p{g}")
                nc.vector.memset(x1p[:, 0:1], 0.0)
                for off, wdt in c.chunks:
                    pa = ps1.tile([128, wdt], F32, tag="pa")
                    pdt = ps1.tile([128, wdt], F32, tag="pdt")
                    for k in range(c.KG):
                        nc.tensor.matmul(
                            pa[:], wag_t[g][:, k * 128:(k + 1) * 128],
                            xt_t[k][:, off:off + wdt],
                            start=(k == 0), stop=(k == c.KG - 1))
                    for k in range(c.KG):
                        nc.tensor.matmul(
                            pdt[:], wdg_t[g][:, k * 128:(k + 1) * 128],
                            xt_t[k][:, off:off + wdt],
                            start=(k == 0), stop=(k == c.KG - 1))
                    sdt = scr.tile([128, wdt], F32, tag="sdt")
                    nc.scalar.activation(sdt[:], pdt[:], AF.Silu,
                                         bias=chpc("sbias", g))
                    # x1 = (a + b_a) * silu(dt + sbias)
                    nc.vector.scalar_tensor_tensor(
                        x1p[:, 1 + off:1 + off + wdt], pa[:],
                        chpc("ba", g), sdt[:], OP.add, OP.mult)
                return x1p

            def convscan(g, x1p):
                # causal depthwise conv k=2 + silu (+ (1-d) prescale)
                sx = pch.tile([128, c.Tw], BF16, tag="sx")
                nc.vector.tensor_scalar(sx[:], x1p[:, 1:], chpc("k1", g),
                                        None, OP.mult)
                ypre = pch.tile([128, c.Tw], BF16, tag="ypre")
                nc.vector.scalar_tensor_tensor(
                    ypre[:], x1p[:, 0:c.Tw], chpc("k0", g), sx[:],
                    OP.mult, OP.add)
                ysl = pch.tile([128, c.Tw], BF16, tag="ysl")
                nc.scalar.activation(ysl[:], ypre[:], AF.Silu)
                u = pch.tile([128, c.Tw], BF16, tag="sx")
                nc.scalar.mul(u[:], ysl[:], chpc("omdf", g))

                # fwd EMA scan, split so token-chunk 0 of the window lands
                # early; first W tokens into discard scratch
                fsl = f_all[:, g * c.Lw:(g + 1) * c.Lw]
                dfb_w = chpc("df", g).broadcast_to([128, c.W])
                dfb_a = chpc("df", g).broadcast_to([128, c.CH])
                dfb_b = chpc("df", g).broadcast_to([128, c.Lw - c.CH])
                fscr = scr.tile([128, c.W], F32, tag="fscr")
                nc.vector.tensor_tensor_scan(
                    fscr[:], dfb_w, u[:, 0:c.W], 0.0, OP.mult, OP.add)
                nc.vector.tensor_tensor_scan(
                    fsl[:, 0:c.CH], dfb_a, u[:, c.W:c.W + c.CH],
                    fscr[:, c.W - 1:c.W], OP.mult, OP.add)

                gsl = slice(g * c.CH, (g + 1) * c.CH)
                st, sp = (g == 0), (g == c.HG - 1)

                def entropy_mms(w, pR, pL):
                    fwsl = f_all[:, g * c.Lw + w * c.CH:
                                 g * c.Lw + (w + 1) * c.CH]
                    nc.scalar.activation(pt_all[w][:, gsl], fwsl, AF.Exp)
                    nc.gpsimd.tensor_tensor(pft_all[w][:, gsl],
                                            pt_all[w][:, gsl], fwsl,
                                            OP.mult)
                    nc.tensor.matmul(pR[0:1, :], ones_b[:],
                                     pt_all[w][:, gsl], start=st, stop=sp)
                    nc.tensor.matmul(pR[32:33, :], ones_b[:],
                                     pft_all[w][:, gsl], start=st, stop=sp)
                    nc.tensor.matmul(pR[64:65, :], wmg_t[g][:],
                                     fwsl, start=st, stop=sp)
                    nc.tensor.matmul(pL[:], wslot_t[g][:],
                                     fwsl, start=st, stop=sp)

                # chunk-0 reductions accumulate inside phase 1
                entropy_mms(0, pR_t[0], pL_t[0])

                nc.vector.tensor_tensor_scan(
                    fsl[:, c.CH:], dfb_b, u[:, c.W + c.CH:c.Tw],
                    fsl[:, c.CH - 1:c.CH], OP.mult, OP.add)
                nc.scalar.activation(
                    pt_all[1][:, gsl],
                    f_all[:, g * c.Lw + c.CH:g * c.Lw + 2 * c.CH], AF.Exp)
                nc.gpsimd.tensor_tensor(
                    pft_all[1][:, gsl], pt_all[1][:, gsl],
                    f_all[:, g * c.Lw + c.CH:g * c.Lw + 2 * c.CH], OP.mult)

                # bwd scan (phase 2, interleaved): warmup slice masked
                gsl = gb_all[:, g * c.Tout:(g + 1) * c.Tout]
                dbb_w = chpc("db", g).broadcast_to([128, c.W])
                dbb_m = chpc("db", g).broadcast_to([128, c.Tout])
                d1m = scr.tile([128, c.W], BF16, tag="d1m")
                nc.vector.scalar_tensor_tensor(
                    d1m[:], fsl[:, c.Tout:c.Lw], chpc("omdb", g),
                    mbw[:], OP.mult, OP.mult)
                d1w = pch.tile([128, c.Tout], BF16, tag="ypre")
                nc.scalar.mul(d1w[:], fsl[:, 0:c.Tout], chpc("omdb", g))
                bscr = scr.tile([128, c.W], F32, tag="bscr")
                nc.vector.tensor_tensor_scan(
                    bscr[:, ::-1], dbb_w, d1m[:, ::-1],
                    0.0, OP.mult, OP.add)
                nc.vector.tensor_tensor_scan(
                    gsl[:, ::-1], dbb_m, d1w[:, ::-1],
                    bscr[:, 0:1], OP.mult, OP.add)

            prev = None
            for g in range(c.HG):
                x1p = inproj(g)
                if prev is not None:
                    convscan(prev[0], prev[1])
                prev = (g, x1p)
            convscan(prev[0], prev[1])

            p1.close()  # release xt/w_in/conv scratch before phase-3 tiles

            # ------------- phase 3: gate scalars, fusion, out-proj ------
            with ExitStack() as p2:
                p3 = p2.enter_context(tc.tile_pool(name="p3", bufs=2))
                pb1 = p2.enter_context(tc.tile_pool(name="pb1", bufs=2))
                row = p2.enter_context(tc.tile_pool(name="row", bufs=2))
                psO = p2.enter_context(tc.tile_pool(name="psO", bufs=3,
                                                    space="PSUM"))
                psD = p2.enter_context(tc.tile_pool(name="psD", bufs=1,
                                                    space="PSUM"))

                # token-chunk 1's channel reductions, deferred to here so
                # phase 1 keeps 6 PSUM banks for in-proj double-buffering;
                # these fill the PE while the last group's scans drain
                pR_t[1] = psD.tile([65, c.CH], F32, tag="pR1", name="pR1")
                pL_t[1] = psD.tile([128, c.CH], F32, tag="pL1", name="pL1")
                for g in range(c.HG):
                    st, sp = (g == 0), (g == c.HG - 1)
                    gsl = slice(g * c.CH, (g + 1) * c.CH)
                    fwsl = f_all[:, g * c.Lw + c.CH:g * c.Lw + 2 * c.CH]
                    nc.tensor.matmul(pR_t[1][0:1, :], ones_b[:],
                                     pt_all[1][:, gsl], start=st, stop=sp)
                    nc.tensor.matmul(pR_t[1][32:33, :], ones_b[:],
                                     pft_all[1][:, gsl], start=st, stop=sp)
                    nc.tensor.matmul(pR_t[1][64:65, :], wmg_t[g][:],
                                     fwsl, start=st, stop=sp)
                    nc.tensor.matmul(pL_t[1][:], wslot_t[g][:],
                                     fwsl, start=st, stop=sp)

                fv = f_all[:].rearrange("p (g l) -> p g l", l=c.Lw)
                gv = gb_all[:].rearrange("p (g l) -> p g l", l=c.Tout)

                # pass P(w): per-token gate scalars + fusion
                wt_t, E2_t = {}, {}
                for w in range(c.WCH):
                    sl = slice(w * c.CH, (w + 1) * c.CH)
                    fw = fv[:, :, sl]       # [128, HG, CH] window view
                    gw = gv[:, :, sl]
                    pR, pL = pR_t[w], pL_t[w]
                    pZ, pG, pM = pR[0:1, :], pR[32:33, :], pR[64:65, :]

                    E = p3.tile([128, c.CH], BF16, tag="E")
                    nc.scalar.activation(E[:], pL[:], AF.Exp, bias=bslot_t[:])
                    # reuse row 0 of the (now dead) slot-logit bank for Zs
                    pZs = pL[0:1, :]
                    nc.tensor.matmul(pZs, ones_b[:], E[:],
                                     start=True, stop=True)

                    # f - gb for the gate-scaled contrast term; no row deps
                    D = p3.tile([128, c.HG * c.CH], BF16, tag="D")
                    Dv = D[:].rearrange("p (g l) -> p g l", l=c.CH)
                    nc.vector.tensor_tensor(Dv, fw, gw, OP.subtract)

                    # per-token gate scalars ([1, CH] rows); sigmoids in
                    # exp/recip form to stay on the natural_log_exp table
                    Zr = row.tile([1, c.CH], F32, tag="Zr")
                    nc.vector.reciprocal_approx_fast(Zr[:], pZ)
                    lnZ = row.tile([1, c.CH], F32, tag="lnZ")
                    nc.scalar.activation(lnZ[:], pZ, AF.Ln)
                    gz = row.tile([1, c.CH], F32, tag="gz")
                    nc.vector.tensor_tensor(gz[:], pG, Zr[:], OP.mult)
                    ent = row.tile([1, c.CH], F32, tag="ent")
                    nc.vector.tensor_tensor(ent[:], lnZ[:], gz[:], OP.subtract)
                    sgq = row.tile([1, c.CH], F32, tag="sgq")
                    nc.scalar.activation(sgq[:], ent[:], AF.Exp,
                                         scale=scc(SC_NSW), bias=scc(SC_NSB))
                    sg1 = row.tile([1, c.CH], F32, tag="sg1")
                    nc.vector.tensor_scalar(sg1[:], sgq[:], 1.0, None, OP.add)
                    gate = row.tile([1, c.CH], F32, tag="gate")
                    nc.vector.reciprocal_approx_fast(gate[:], sg1[:])
                    mgq = row.tile([1, c.CH], F32, tag="mgq")
                    nc.scalar.activation(mgq[:], pM, AF.Exp,
                                         scale=-1.0, bias=scc(SC_NBMG))
                    mg1 = row.tile([1, c.CH], F32, tag="mg1")
                    nc.vector.tensor_scalar(mg1[:], mgq[:], 1.0, None, OP.add)
                    mgi = row.tile([1, c.CH], F32, tag="mgi")
                    nc.vector.reciprocal_approx_fast(mgi[:], mg1[:])
                    Zsr = row.tile([1, c.CH], F32, tag="Zsr")
                    nc.vector.reciprocal_approx_fast(Zsr[:], pZs)
                    # pack GB=f1*gate | s2 into one row, bounce via DRAM to
                    # broadcast across partitions (DMA only)
                    abc = row.tile([1, 2 * c.CH], BF16, tag="abc")
                    nc.vector.tensor_scalar(abc[:, 0:c.CH], gate[:],
                                            scc(SC_F1), None, OP.mult)
                    nc.vector.scalar_tensor_tensor(
                        abc[:, c.CH:2 * c.CH], mgi[:], scc(SC_F2),
                        Zsr[:], OP.mult, OP.mult)
                    nc.sync.dma_start(rowbc[w:w + 1, :], abc[:])
                    ABC = pb1.tile([128, 2 * c.CH], BF16, tag="ABC")
                    nc.sync.dma_start(
                        ABC[:], rowbc[w:w + 1, :].broadcast_to([128, 2 * c.CH]))
                    GBB = ABC[:, 0:c.CH]
                    S2B = ABC[:, c.CH:2 * c.CH]

                    E2 = p3.tile([128, c.CH], BF16, tag="E2")
                    nc.vector.tensor_tensor(E2[:], E[:], S2B, OP.mult)
                    E2_t[w] = E2

                    # weighted = f0*f + (f1*gate)*(f - gb), in halves so the
                    # out-proj k-loop can start on the first half; memory
                    # read is folded into out-proj
                    GBv = GBB.rearrange("p (o l) -> p o l", o=1) \
                        .broadcast_to([128, c.HG // 2, c.CH])
                    wtp = p3.tile([128, c.HG * c.CH], BF16, tag="wtp")
                    wtpv = wtp[:].rearrange("p (g l) -> p g l", l=c.CH)
                    wt = p3.tile([128, c.HG * c.CH], BF16, tag="wt")
                    wtv = wt[:].rearrange("p (g l) -> p g l", l=c.CH)
                    hh = c.HG // 2
                    for h in (slice(0, hh), slice(hh, c.HG)):
                        nc.vector.tensor_tensor(wtpv[:, h], Dv[:, h], GBv,
                                                OP.mult)
                        nc.vector.scalar_tensor_tensor(
                            wtv[:, h], fw[:, h], sccB(SC_F0), wtpv[:, h],
                            OP.mult, OP.add)
                    wt_t[w] = wt

                # pass Q(w): out-proj (+ folded memory read, WM last so E2
                # has time to land while the wout accumulation streams)
                for w in range(c.WCH):
                    sl = slice(w * c.CH, (w + 1) * c.CH)
                    wt, E2 = wt_t[w], E2_t[w]
                    for m in range(c.OG):
                        mc = slice(m * 128, (m + 1) * 128)
                        po = psO.tile([128, c.CH], F32, tag="po")
                        for k in range(c.HG):
                            nc.tensor.matmul(
                                po[:], wout_t[k][:, mc],
                                wt[:, k * c.CH:(k + 1) * c.CH],
                                start=(k == 0), stop=False)
                        nc.tensor.matmul(po[:], wm_t[:, mc], E2[:],
                                         start=False, stop=True)
                        ob = p3.tile([128, c.CH], F32, tag="ob")
                        nc.scalar.activation(ob[:], po[:], AF.Identity,
                                             bias=bout_t[:, m:m + 1])
                        nc.sync.dma_start(out_d[m * 128:(m + 1) * 128, sl], ob[:])

    nc.compile()
    _canon_act_tables(nc)
    return nc


_PROG_CACHE = {}


def _get_prog(cfg: Cfg):
    key = (cfg.DI, cfg.H, cfg.O, cfg.S, cfg.T, cfg.W, cfg.CH)
    if key not in _PROG_CACHE:
        _PROG_CACHE[key] = build_program(cfg)
    return _PROG_CACHE[key]


def make_in_maps(cfg, x, W_in, b_in, dt_bias_fwd, conv_k, decay_fwd, decay_bwd,
                 memory, mem_decay, W_mem_gate, b_mem_gate, W_slot, b_slot,
                 W_slot_bwd, b_slot_bwd, fusion_weight, scaler_w, scaler_b,
                 W_out, b_out):
    c = cfg
    x = np.asarray(x)
    B, T, DI = x.shape
    f32 = np.float32

    def sig(v):
        return 1.0 / (1.0 + np.exp(-np.asarray(v, np.float64)))

    def col(v):  # [H] -> [128, HG] column blocks
        return np.ascontiguousarray(np.asarray(v, f32).reshape(c.HG, 128).T)

    df = sig(decay_fwd)
    db = sig(decay_bwd)
    chp = np.concatenate([
        col(conv_k[:, 0]), col(conv_k[:, 1]),
        col((1.0 - df)), col(df),
        col((1.0 - db)), col(db),
        col(np.asarray(b_in)[c.H:] + np.asarray(dt_bias_fwd)),
        col(np.asarray(b_in)[:c.H]),
    ], axis=1).astype(f32)
    bout = np.ascontiguousarray(np.asarray(b_out, f32).reshape(c.OG, 128).T)
    bslot = np.asarray(b_slot_bwd, f32).reshape(128, 1)
    scv = np.zeros((1, 8), f32)
    scv[0, SC_F1] = fusion_weight[1]
    scv[0, SC_F0] = fusion_weight[0]
    scv[0, SC_NF1] = -fusion_weight[1]
    scv[0, SC_F2] = fusion_weight[2]
    scv[0, SC_NSW] = -scaler_w[0]
    scv[0, SC_NSB] = -scaler_b[0]
    scv[0, SC_NBMG] = -b_mem_gate[0]
    mem_scaled = np.asarray(memory, f32) * sig(mem_decay)[:, None].astype(f32)
    wm = (mem_scaled.astype(f32) @ np.asarray(W_out, f32)).astype(BF16_NP)

    shared = {
        "w_in": np.ascontiguousarray(np.asarray(W_in, f32).astype(BF16_NP)),
        "w_out": np.ascontiguousarray(np.asarray(W_out, f32).astype(BF16_NP)),
        "w_slot": np.ascontiguousarray(np.asarray(W_slot_bwd, f32).astype(BF16_NP)),
        "w_mg": np.ascontiguousarray(np.asarray(W_mem_gate, f32).astype(BF16_NP)),
        "wm": np.ascontiguousarray(wm),
        "chp": chp, "bout": bout, "bslot": bslot, "sc": scv,
    }
    in_maps = []
    for core in range(8):
        b, j = divmod(core, 2)
        start = j * c.Tout - c.W
        gs, ge = max(0, start), min(T, start + c.Tw)
        xt = np.zeros((c.DI, c.Tw), BF16_NP)
        xt[:, gs - start:ge - start] = x[b, gs:ge, :].T.astype(BF16_NP)
        # mask for the bwd-scan warmup slice (local tokens [W+Tout, Tw)):
        # 1 where the token is a real sequence position, 0 in the zero-pad
        mask = np.zeros((1, c.W), BF16_NP)
        ws, we = c.W + c.Tout, c.Tw  # local token range of the warmup slice
        vs, ve = gs - start, ge - start  # valid local token range
        ov_s, ov_e = max(ws, vs), min(we, ve)
        if ov_s < ov_e:
            mask[0, ov_s - ws:ov_e - ws] = 1.0
        m = dict(shared)
        m["xt"] = xt
        m["mask"] = mask
        in_maps.append(m)
    return in_maps


def run(cfg, inputs, trace=False, tmpdir=None):
    nc = _get_prog(cfg)
    in_maps = make_in_maps(cfg, **inputs)
    res = run_bass_kernel_spmd(nc, in_maps, core_ids=list(range(8)),
                               trace=trace, tmpdir=tmpdir)
    B, T = np.asarray(inputs["x"]).shape[0], np.asarray(inputs["x"]).shape[1]
    out = np.empty((B, T, cfg.O), np.float32)
    for core in range(8):
        b, j = divmod(core, 2)
        out[b, j * cfg.Tout:(j + 1) * cfg.Tout, :] = res.results[core]["outT"].T
    return out, res


def kernel(**inputs):
    cfg = Cfg()
    out, _ = run(cfg, inputs)
    return out
